# revision 10
# baseline (speedup 1.0000x reference)
"""Trainium2 Bass kernel for Chapter4WindPowerModel (segment_reduce).

Sharding: pure data-parallel over B (B=8 == n_cores): core b computes batch
element b entirely (decomposition + 3 encoder branches + heads) with no
cross-core communication; outputs are stacked on the host.

Per-core structure:
  - x[b] [14,100,576] is loaded into "ribbon" SBUF tiles: 4 row-groups at
    partitions 0/32/64/96 (14 channels each), 5 series per row, each series
    padded with 25 zero columns so the 25-tap moving average (trend) is a
    pure chain of shifted adds with exact zero-padding semantics.
  - trend/daily/hf are computed on the Vector engine in this layout.
  - conv1 (1x3, 14->128) runs as 3 tap-matmuls (K=14) per 288-column chunk
    directly from the ribbons using PE row-group tiling (tile_position).
  - conv2 (1x3, 128->128) runs as 3 tap-matmuls (K=128) from the gelu1
    output; both convs accumulate in PSUM, float32r at full PE rate.
  - gelu on ScalarE with fused bias; the second gelu also emits accum_out
    (per-partition sum over (n,t)) used for the projection-head mean.
  - Predictor/gate/softmax/proj run once per core on the staged last-12
    columns ([128, 1200] per branch).
"""
import numpy as np
from contextlib import ExitStack

import concourse.bass as bass
import concourse.tile as tile
from concourse import bacc, mybir
from concourse.bass_utils import run_bass_kernel_spmd

f32 = mybir.dt.float32
f32r = mybir.dt.float32r
bf16 = mybir.dt.bfloat16
AF = mybir.ActivationFunctionType
ALU = mybir.AluOpType
AX = mybir.AxisListType

IN_DIM, HID, HORIZON, PERIOD, TREND_K = 14, 128, 12, 144, 25
B, N, T = 8, 100, 576
NB2 = 3 * HID  # 384

# ribbon layout
L = 5                      # series per ribbon row
ROWS = 4                   # row groups at partitions 0/32/64/96
NPR = ROWS * L             # 20 series per round
ROUNDS = N // NPR          # 5
GAP = 25                   # zero gap >= TREND_K
RST = T + GAP              # 601 stride per series
WRIB = GAP + L * RST       # 3030
CH = 288                   # T chunk (2 chunks per series, <=512 psum bank)
BR = ("hf", "daily", "trend")


def _data_ap(t, k, ndim3=False):
    """AP over the data columns of ribbon row k: free dims (L, 601),(576)."""
    a = t[32 * k:32 * k + 14, GAP:GAP + L * RST]
    a = a.rearrange("p (n t) -> p n t", n=L)[:, :, 0:T]
    if ndim3:
        a = a.rearrange("p n (q c) -> p n q c", q=4)
    return a


def build():
    nc = bacc.Bacc("TRN2", target_bir_lowering=False, debug=False,
                   enable_asserts=False)

    X = nc.dram_tensor("x", [IN_DIM, N, T], f32, kind="ExternalInput")
    dram = {}
    for b in BR:
        dram[f"{b}_enc_w1"] = nc.dram_tensor(f"{b}_enc_w1", [HID, IN_DIM, 1, 3], f32r, kind="ExternalInput")
        dram[f"{b}_enc_b1"] = nc.dram_tensor(f"{b}_enc_b1", [HID], f32, kind="ExternalInput")
        dram[f"{b}_enc_w2"] = nc.dram_tensor(f"{b}_enc_w2", [HID, HID, 1, 3], f32, kind="ExternalInput")
        dram[f"{b}_enc_b2"] = nc.dram_tensor(f"{b}_enc_b2", [HID], f32, kind="ExternalInput")
        dram[f"{b}_pred_w1"] = nc.dram_tensor(f"{b}_pred_w1", [64, HID, 1, 1], f32r, kind="ExternalInput")
        dram[f"{b}_pred_b1"] = nc.dram_tensor(f"{b}_pred_b1", [64], f32, kind="ExternalInput")
        dram[f"{b}_pred_w2"] = nc.dram_tensor(f"{b}_pred_w2", [1, 64, 1, 1], f32, kind="ExternalInput")
        dram[f"{b}_pred_b2"] = nc.dram_tensor(f"{b}_pred_b2", [1], f32, kind="ExternalInput")
    dram["gate_w1"] = nc.dram_tensor("gate_w1", [HID, NB2, 1, 1], f32r, kind="ExternalInput")
    dram["gate_b1"] = nc.dram_tensor("gate_b1", [HID], f32, kind="ExternalInput")
    dram["gate_w2"] = nc.dram_tensor("gate_w2", [3, HID, 1, 1], f32r, kind="ExternalInput")
    dram["gate_b2"] = nc.dram_tensor("gate_b2", [3], f32, kind="ExternalInput")
    dram["proj_w1"] = nc.dram_tensor("proj_w1", [NB2, NB2], f32, kind="ExternalInput")
    dram["proj_b1"] = nc.dram_tensor("proj_b1", [NB2], f32, kind="ExternalInput")
    dram["proj_w2"] = nc.dram_tensor("proj_w2", [64, NB2], f32, kind="ExternalInput")
    dram["proj_b2"] = nc.dram_tensor("proj_b2", [64], f32, kind="ExternalInput")

    PRED = nc.dram_tensor("pred", [N, HORIZON], f32, kind="ExternalOutput")
    REP = nc.dram_tensor("rep", [64], f32, kind="ExternalOutput")

    with tile.TileContext(nc) as tc, ExitStack() as ctx:
        wp = ctx.enter_context(tc.tile_pool(name="wp", bufs=1))
        const_p = ctx.enter_context(tc.tile_pool(name="const", bufs=1))
        rib = ctx.enter_context(tc.tile_pool(name="rib", bufs=7))
        pmp = ctx.enter_context(tc.tile_pool(name="pm", bufs=2))
        h1p = ctx.enter_context(tc.tile_pool(name="h1", bufs=2))
        h2p = ctx.enter_context(tc.tile_pool(name="h2", bufs=2))
        outp = ctx.enter_context(tc.tile_pool(name="outp", bufs=1))

        # ---------------- weights staging ----------------
        w1t = {}   # (branch, tap) -> [128,128] tile, 4 row-replicas of W1tap.T
        w2t = {}   # (branch, tap) -> [128,128] W2tap.T
        b1t, b2t = {}, {}
        for b in BR:
            for kk in range(3):
                t1 = wp.tile([128, 128], f32r, tag=f"w1_{b}_{kk}")
                for k in range(ROWS):
                    nc.sync.dma_start(
                        t1[32 * k:32 * k + 14, :],
                        dram[f"{b}_enc_w1"].ap()[:, :, 0, kk].rearrange("o c -> c o"))
                w1t[(b, kk)] = t1
                t2s = wp.tile([128, 128], f32, tag="wscr", bufs=2, name=f"w2s_{b}_{kk}")
                nc.sync.dma_start(
                    t2s[:], dram[f"{b}_enc_w2"].ap()[:, :, 0, kk].rearrange("o c -> c o"))
                t2 = wp.tile([128, 128], bf16, tag=f"w2_{b}_{kk}")
                nc.vector.tensor_copy(t2[:], t2s[:])
                w2t[(b, kk)] = t2
            bt = wp.tile([128, 1], f32, tag=f"b1_{b}")
            nc.sync.dma_start(bt[:, 0:1], dram[f"{b}_enc_b1"].ap().rearrange("(o u) -> o u", u=1))
            b1t[b] = bt
            bt2 = wp.tile([128, 1], f32, tag=f"b2_{b}")
            nc.sync.dma_start(bt2[:, 0:1], dram[f"{b}_enc_b2"].ap().rearrange("(o u) -> o u", u=1))
            b2t[b] = bt2

        pw1t, pb1t, pw2t, pb2t = {}, {}, {}, {}
        for b in BR:
            t = wp.tile([128, 64], f32r, tag=f"pw1_{b}")
            nc.sync.dma_start(t[:], dram[f"{b}_pred_w1"].ap()[:, :, 0, 0].rearrange("o c -> c o"))
            pw1t[b] = t
            t = wp.tile([64, 1], f32, tag=f"pb1_{b}")
            nc.sync.dma_start(t[:, 0:1], dram[f"{b}_pred_b1"].ap().rearrange("(o u) -> o u", u=1))
            pb1t[b] = t
            ts_ = wp.tile([128, 1], f32, tag="wscr2", bufs=2, name=f"pw2s_{b}")
            nc.sync.dma_start(ts_[0:64, 0:1],
                              dram[f"{b}_pred_w2"].ap()[:, :, 0, 0].rearrange("o c -> c o"))
            nc.sync.dma_start(ts_[64:65, 0:1],
                              dram[f"{b}_pred_b2"].ap().rearrange("(o u) -> o u", u=1))
            t = wp.tile([128, 3], bf16, tag=f"pw2_{b}", name=f"pw2b_{b}")
            nc.gpsimd.memset(t[:], 0.0)
            bi_ = BR.index(b)
            nc.vector.tensor_copy(t[0:65, bi_:bi_ + 1], ts_[0:65, 0:1])
            pw2t[b] = t

        gw1t = wp.tile([128, 3 * 128], f32r, tag="gw1")
        for j in range(3):
            nc.sync.dma_start(
                gw1t[:, j * 128:(j + 1) * 128],
                dram["gate_w1"].ap()[:, j * 128:(j + 1) * 128, 0, 0].rearrange("o c -> c o"))
        gb1t = wp.tile([128, 1], f32, tag="gb1")
        nc.sync.dma_start(gb1t[:, 0:1], dram["gate_b1"].ap().rearrange("(o u) -> o u", u=1))
        gw2t = wp.tile([128, 3], f32r, tag="gw2")
        nc.sync.dma_start(gw2t[:, 0:3], dram["gate_w2"].ap()[:, :, 0, 0].rearrange("o c -> c o"))
        gb2t = wp.tile([3, 1], f32, tag="gb2")
        nc.sync.dma_start(gb2t[0:3, 0:1], dram["gate_b2"].ap().rearrange("(o u) -> o u", u=1))

        pj1s = wp.tile([128, 9 * 128], f32, tag="pj1s")
        for mj in range(3):
            for kj in range(3):
                nc.sync.dma_start(
                    pj1s[:, (mj * 3 + kj) * 128:(mj * 3 + kj + 1) * 128],
                    dram["proj_w1"].ap()[mj * 128:(mj + 1) * 128,
                                         kj * 128:(kj + 1) * 128].rearrange("m k -> k m"))
        pj1 = wp.tile([128, 9 * 128], bf16, tag="pj1")
        nc.vector.tensor_copy(pj1[:], pj1s[:])
        pjb1 = wp.tile([128, 3], f32, tag="pjb1")
        for mj in range(3):
            nc.sync.dma_start(
                pjb1[:, mj:mj + 1],
                dram["proj_b1"].ap()[mj * 128:(mj + 1) * 128].rearrange("(o u) -> o u", u=1))
        pj2s = wp.tile([128, 3 * 64], f32, tag="pj2s")
        for kj in range(3):
            nc.sync.dma_start(
                pj2s[:, kj * 64:(kj + 1) * 64],
                dram["proj_w2"].ap()[:, kj * 128:(kj + 1) * 128].rearrange("m k -> k m"))
        pj2 = wp.tile([128, 3 * 64], bf16, tag="pj2")
        nc.vector.tensor_copy(pj2[:], pj2s[:])
        pjb2 = wp.tile([64, 1], f32, tag="pjb2")
        nc.sync.dma_start(pjb2[:, 0:1], dram["proj_b2"].ap().rearrange("(o u) -> o u", u=1))

        ones3 = const_p.tile([3, 1], bf16, tag="ones3")
        nc.gpsimd.memset(ones3[:], 1.0)
        ones64 = const_p.tile([64, 1], bf16, tag="ones64")
        nc.gpsimd.memset(ones64[:], 1.0)
        ones1x64 = const_p.tile([1, 64], bf16, tag="ones1x64")
        nc.gpsimd.memset(ones1x64[:], 1.0)

        # persistent accumulators / staging
        accs = {b: outp.tile([128, 64], f32, tag=f"accs_{b}", name=f"accs_{b}") for b in BR}
        pstage = {b: outp.tile([128, N * HORIZON], f32r, tag=f"pst_{b}", name=f"pstage_{b}") for b in BR}

        # ---------------- main loop ----------------
        main_ctx = ExitStack()
        psm = main_ctx.enter_context(tc.tile_pool(name="psm", bufs=2, space="PSUM"))
        for r in range(ROUNDS):
            # x ribbon
            xt = rib.tile([128, WRIB], f32, tag="rib")
            nc.gpsimd.memset(xt[:], 0.0)
            for k in range(ROWS):
                n0 = r * NPR + k * L
                nc.sync.dma_start(_data_ap(xt, k), X.ap()[:, n0:n0 + L, :])

            W = WRIB
            s2 = rib.tile([128, W], f32, tag="rib")
            nc.vector.tensor_add(s2[:, 0:W - 2], xt[:, 0:W - 2], xt[:, 1:W - 1])
            s4 = rib.tile([128, W], f32, tag="rib")
            nc.vector.tensor_add(s4[:, 0:W - 4], s2[:, 0:W - 4], s2[:, 2:W - 2])
            s5 = rib.tile([128, W], f32, tag="rib")
            nc.vector.tensor_add(s5[:, 0:W - 5], s4[:, 0:W - 5], xt[:, 4:W - 1])
            s10 = rib.tile([128, W], f32, tag="rib")
            nc.vector.tensor_add(s10[:, 0:W - 10], s5[:, 0:W - 10], s5[:, 5:W - 5])
            s20 = rib.tile([128, W], f32, tag="rib")
            nc.vector.tensor_add(s20[:, 0:W - 20], s10[:, 0:W - 20], s10[:, 10:W - 10])
            s25 = rib.tile([128, W], f32, tag="rib")
            nc.vector.tensor_add(s25[:, 0:W - 25], s20[:, 0:W - 25], s5[:, 20:W - 5])

            # per-phase mean (daily basis): pm = 0.25*(x0+x1+x2+x3) without scale
            pm1 = pmp.tile([128, L * PERIOD], f32, tag="pm1")
            pm2 = pmp.tile([128, L * PERIOD], f32, tag="pm2")
            pm4 = pmp.tile([128, L * PERIOD], f32, tag="pm4")

            def xper(k, per):
                a = xt[32 * k:32 * k + 14, GAP:GAP + L * RST]
                a = a.rearrange("p (n t) -> p n t", n=L)
                return a[:, :, per * PERIOD:(per + 1) * PERIOD]

            for k in range(ROWS):
                pm1v = pm1[32 * k:32 * k + 14, :].rearrange("p (n t) -> p n t", n=L)
                pm2v = pm2[32 * k:32 * k + 14, :].rearrange("p (n t) -> p n t", n=L)
                pm4v = pm4[32 * k:32 * k + 14, :].rearrange("p (n t) -> p n t", n=L)
                nc.vector.tensor_add(pm1v, xper(k, 0), xper(k, 1))
                nc.vector.tensor_add(pm2v, xper(k, 2), xper(k, 3))
                nc.vector.tensor_add(pm4v, pm1v, pm2v)

            trend = rib.tile([128, W], f32r, tag="rib")
            hft = rib.tile([128, W], f32r, tag="rib")
            daily = rib.tile([128, W], f32r, tag="rib")
            for k in range(ROWS):
                s25d = s25[32 * k:32 * k + 14, GAP - 12:GAP - 12 + L * RST]
                s25d = s25d.rearrange("p (n t) -> p n t", n=L)[:, :, 0:T]
                # trend = s25 / 25  (shifted -12)
                nc.vector.tensor_scalar_mul(_data_ap(trend, k), s25d, 1.0 / TREND_K)
                # t1 = x - 0.5*trend = x - s25/50
                nc.vector.scalar_tensor_tensor(
                    _data_ap(hft, k), s25d, -1.0 / (2 * TREND_K), _data_ap(xt, k),
                    op0=ALU.mult, op1=ALU.add)
                # per-series ops: walrus caps DVE APs at partition + 2 free dims
                for j in range(L):
                    pmb = pm4[32 * k:32 * k + 14, j * PERIOD:(j + 1) * PERIOD]
                    pmb = pmb.rearrange("p (u q) -> p u q", u=1)
                    pmb = pmb.broadcast_to([14, 4, PERIOD])
                    cn = GAP + j * RST
                    dv = daily[32 * k:32 * k + 14, cn:cn + T].rearrange(
                        "p (u q) -> p u q", u=4)
                    hv = hft[32 * k:32 * k + 14, cn:cn + T].rearrange(
                        "p (u q) -> p u q", u=4)
                    # daily = pm4 * 0.25 (tiled 4x)
                    nc.vector.tensor_scalar_mul(dv, pmb, 0.25)
                    # hf = t1 - 0.125 * pm4(tiled)
                    nc.vector.scalar_tensor_tensor(
                        hv, pmb, -0.125, hv.bitcast(f32), op0=ALU.mult, op1=ALU.add)

            # zero the per-series guard columns (read by conv1 full-width taps)
            for rbt in (trend, hft, daily):
                gv = rbt[:, GAP - 1:GAP - 1 + L * RST].rearrange(
                    "p (n g) -> p n g", n=L)[:, :, 0:578:577]
                xv = xt[:, GAP - 1:GAP - 1 + L * RST].rearrange(
                    "p (n g) -> p n g", n=L)[:, :, 0:578:577]
                nc.vector.tensor_copy(gv, xv)

            ribs = {"hf": hft, "daily": daily, "trend": trend}

            # conv phase: pairs of consecutive series
            for pi in range(NPR // 2):
                p_glob = r * (NPR // 2) + pi
                nn = [2 * pi, 2 * pi + 1]            # series index within round
                slots = [(q // L, q % L) for q in nn]  # (row k, j)
                for b in BR:
                    rb = ribs[b]
                    P1 = psm.tile([128, 2048], f32, tag="ps")
                    for s, (k, j) in enumerate(slots):
                        cn = GAP + j * RST
                        for c in range(2):
                            t0 = CH * c
                            for kk in (1, 0, 2):  # middle tap first (start=True)
                                d = kk - 1
                                # full-width taps: edge reads land in the
                                # zeroed ribbon guard columns (exact zero-pad)
                                nc.tensor.matmul(
                                    P1[:, s * 1024 + c * 512: s * 1024 + c * 512 + CH],
                                    w1t[(b, kk)][32 * k:32 * k + 14, :],
                                    rb[32 * k:32 * k + 14, cn + t0 + d: cn + t0 + d + CH],
                                    start=(kk == 1), stop=(kk == 2),
                                    tile_position=(32 * k, 0))
                    h1 = h1p.tile([128, 4 * CH], bf16, tag="h1")
                    nc.scalar.activation(
                        h1[:].rearrange("p (q c) -> p q c", q=4),
                        P1[:].rearrange("p (q c) -> p q c", q=4)[:, :, 0:CH],
                        AF.Gelu, bias=b1t[b][:, 0:1])

                    P2 = psm.tile([128, 2048], f32, tag="ps")
                    for s in range(2):
                        hb = s * 576
                        for c in range(2):
                            t0 = CH * c
                            for kk in (1, 0, 2):
                                d = kk - 1
                                if d == -1:
                                    lo, hi = max(t0, 1), t0 + CH
                                elif d == 1:
                                    lo, hi = t0, min(t0 + CH, T - 1)
                                else:
                                    lo, hi = t0, t0 + CH
                                nc.tensor.matmul(
                                    P2[:, s * 1024 + c * 512 + (lo - t0): s * 1024 + c * 512 + (hi - t0)],
                                    w2t[(b, kk)][:],
                                    h1[:, hb + lo + d: hb + hi + d],
                                    start=(kk == 1), stop=(kk == 2))
                    h2 = h2p.tile([128, 4 * CH], f32, tag="h2")
                    nc.scalar.activation(
                        h2[:].rearrange("p (q c) -> p q c", q=4),
                        P2[:].rearrange("p (q c) -> p q c", q=4)[:, :, 0:CH],
                        AF.Gelu, bias=b2t[b][:, 0:1],
                        accum_out=accs[b][:, p_glob:p_glob + 1])
                    nc.vector.tensor_copy(
                        pstage[b][:, p_glob * 24:(p_glob + 1) * 24].rearrange("p (n c) -> p n c", n=2),
                        h2[:].rearrange("p (n t) -> p n t", n=2)[:, :, T - HORIZON:T])

        # ---------------- tail ----------------
        main_ctx.close()
        pst = ctx.enter_context(tc.tile_pool(name="pst", bufs=2, space="PSUM"))
        tbp = ctx.enter_context(tc.tile_pool(name="tbp", bufs=6))
        NP_ALL = N // 2  # 50 pairs
        NT = N * HORIZON  # 1200
        chunks = [(0, 512), (512, 1024), (1024, NT)]

        zt = outp.tile([128, 3], bf16, tag="zt")
        for bi, b in enumerate(BR):
            zs = outp.tile([128, 1], f32, tag=f"zs_{b}")
            nc.vector.reduce_sum(zs[:, 0:1], accs[b][:, 0:NP_ALL], axis=AX.X)
            nc.vector.tensor_scalar_mul(zt[:, bi:bi + 1], zs[:, 0:1], 1.0 / (N * T))

        # proj head
        zp = pst.tile([128, 3], f32, tag="pst")
        for mj in range(3):
            for kj in range(3):
                nc.tensor.matmul(
                    zp[:, mj:mj + 1],
                    pj1[:, (mj * 3 + kj) * 128:(mj * 3 + kj + 1) * 128],
                    zt[:, kj:kj + 1],
                    start=(kj == 0), stop=(kj == 2))
        z1 = outp.tile([128, 3], bf16, tag="z1")
        for mj in range(3):
            nc.scalar.activation(z1[:, mj:mj + 1], zp[:, mj:mj + 1],
                                 AF.Gelu, bias=pjb1[:, mj:mj + 1])
        rp = pst.tile([64, 1], f32, tag="pst")
        for kj in range(3):
            nc.tensor.matmul(rp[:, 0:1], pj2[:, kj * 64:(kj + 1) * 64],
                             z1[:, kj:kj + 1],
                             start=(kj == 0), stop=(kj == 2))
        rep_sb = outp.tile([64, 1], f32, tag="rep_sb")
        nc.vector.tensor_scalar_add(rep_sb[:, 0:1], rp[:, 0:1], pjb2[:, 0:1])
        sq = outp.tile([64, 1], bf16, tag="sq")
        nc.vector.tensor_mul(sq[:, 0:1], rep_sb[:, 0:1], rep_sb[:, 0:1])
        ssq = pst.tile([1, 1], f32, tag="pst")
        nc.tensor.matmul(ssq[0:1, 0:1], ones64[:], sq[:, 0:1],
                         start=True, stop=True)
        nrm = outp.tile([1, 1], f32, tag="nrm")
        nc.scalar.activation(nrm[0:1, 0:1], ssq[0:1, 0:1], AF.Sqrt)
        nmx = outp.tile([1, 1], f32, tag="nmx")
        nc.vector.tensor_scalar_max(nmx[0:1, 0:1], nrm[0:1, 0:1], 1e-12)
        inv = outp.tile([1, 1], bf16, tag="inv")
        with nc.allow_low_precision(reason="1/norm broadcast via f32r matmul"):
            nc.vector.reciprocal(inv[0:1, 0:1], nmx[0:1, 0:1])
        binv = pst.tile([64, 1], f32, tag="pst")
        nc.tensor.matmul(binv[:, 0:1], ones1x64[:], inv[0:1, 0:1],
                         start=True, stop=True)
        rep_out = outp.tile([64, 1], f32, tag="rep_out")
        nc.vector.tensor_mul(rep_out[:, 0:1], rep_sb[:, 0:1], binv[:, 0:1])
        nc.sync.dma_start(REP.ap(), rep_out[:, 0:1])

        # predictor heads: ystack[b,:] = w2_b . gelu(w1_b . H + b1) + b2_b
        # (bias folded in via a ones-row: K=65 matmul with zero-padded lhsT)
        ystack = pst.tile([3, NT], f32, tag="pst")
        for bi, b in enumerate(BR):
            y1p = pst.tile([64, NT], f32, tag="pst")
            for lo, hi in chunks:
                nc.tensor.matmul(y1p[:, lo:hi], pw1t[b][:],
                                 pstage[b][:, lo:hi], start=True, stop=True)
            ya = tbp.tile([128, NT], bf16, tag="tail")
            nc.scalar.activation(ya[0:64, :], y1p[:], AF.Gelu, bias=pb1t[b][:, 0:1])
            nc.gpsimd.memset(ya[64:65, :], 1.0)
            for lo, hi in chunks:
                nc.tensor.matmul(ystack[0:3, lo:hi], pw2t[b][0:65, 0:3],
                                 ya[0:65, lo:hi],
                                 start=(bi == 0), stop=(bi == 2))

        ysb = tbp.tile([4, NT], f32, tag="tail")
        nc.scalar.copy(ysb[0:3, :], ystack[0:3, :])

        gp1 = pst.tile([128, NT], f32, tag="pst")
        for lo, hi in chunks:
            for bi, b in enumerate(BR):
                nc.tensor.matmul(gp1[:, lo:hi], gw1t[:, bi * 128:(bi + 1) * 128],
                                 pstage[b][:, lo:hi],
                                 start=(bi == 0), stop=(bi == 2))
        ga = tbp.tile([128, NT], f32r, tag="tail")
        nc.scalar.activation(ga[:], gp1[:], AF.Gelu, bias=gb1t[:, 0:1])
        gp2 = pst.tile([3, NT], f32, tag="pst")
        for lo, hi in chunks:
            nc.tensor.matmul(gp2[0:3, lo:hi], gw2t[:, 0:3],
                             ga[:, lo:hi], start=True, stop=True)
        es = tbp.tile([4, NT], bf16, tag="tail")
        nc.scalar.activation(es[0:3, :], gp2[0:3, :], AF.Exp, bias=gb2t[0:3, 0:1])

        # weighted sum / softmax via PE partition-sums (ones3 contraction)
        ps_ = tbp.tile([4, NT], bf16, tag="tail")
        nc.vector.tensor_mul(ps_[0:3, :], es[0:3, :], ysb[0:3, :])
        nd = pst.tile([2, NT], f32, tag="pst")
        for lo, hi in chunks:
            nc.tensor.matmul(nd[0:1, lo:hi], ones3[:],
                             ps_[0:3, lo:hi], start=True, stop=True)
        es2 = pst.tile([1, NT], f32, tag="pst")
        for lo, hi in chunks:
            nc.tensor.matmul(es2[0:1, lo:hi], ones3[:],
                             es[0:3, lo:hi], start=True, stop=True)
        invd = tbp.tile([1, NT], f32, tag="tail")
        nc.vector.reciprocal(invd[0:1, :], es2[0:1, :])
        predv = tbp.tile([1, NT], f32, tag="tail")
        nc.vector.tensor_mul(predv[0:1, :], nd[0:1, :], invd[0:1, :])
        nc.sync.dma_start(PRED.ap(), predv[0:1, :])

    nc.compile()
    return nc


_NC = None


def _get_nc():
    global _NC
    if _NC is None:
        _NC = build()
    return _NC


def _flat_weights(params):
    out = {}
    for b in BR:
        for part, pp in (("enc", f"{b}_enc"), ("pred", f"{b}_pred")):
            src = params[f"{b}_{part}"]
            for wn in ("w1", "b1", "w2", "b2"):
                out[f"{pp}_{wn}"] = np.asarray(src[wn], dtype=np.float32)
    for grp in ("gate", "proj"):
        for wn in ("w1", "b1", "w2", "b2"):
            out[f"{grp}_{wn}"] = np.asarray(params[grp][wn], dtype=np.float32)
    return out


def kernel(x, params):
    x = np.asarray(x, dtype=np.float32)          # [8,14,100,576]
    w = _flat_weights(params)
    nc = _get_nc()
    in_maps = [{"x": np.ascontiguousarray(x[i]), **w} for i in range(B)]
    res = run_bass_kernel_spmd(nc, in_maps, core_ids=list(range(B)))
    preds = np.stack([res.results[i]["pred"] for i in range(B)])  # [8,100,12]
    reps = np.stack([res.results[i]["rep"] for i in range(B)])    # [8,64]
    return preds[:, None, :, :], reps
